# revision 4
# baseline (speedup 1.0000x reference)
"""NodeDropout kernel for 8 trn2 NeuronCores.

out[e] = values[e] * keep[src[e]] * keep[dst[e]],  keep = ~nodes_flag (1M bools).

Device algorithm (edges sharded 8 ways data-parallel, per core):
- keep bit-packed host-side into a 31250-word uint32 table (1M bits),
  replicated into every SBUF partition (~122KB/partition).
- gpsimd.ap_gather: each 16-partition group consumes one shared int16 index
  stream; out[p, i] = table[p, stream_{p//16}[i]] (replicated within a group
  since the table is replicated).
- Per batch of B = 49152 edges: one fused gather stream per group =
  [per-edge dst word idxs (6144)] ++ [src cluster word idxs (384)].
  Host sorts edges by src word and forms clusters of K=16 edges sharing one
  src word, so src lookups cost 1/16 per edge.
- Compaction: the gather output [128, L] is 16x redundant; a plain DMA from
  the strided-partition view [8, L] (partitions 0,16,..,112) reshaped to
  [128, L/16] lands chunk r of group c's stream on partition 16c+r - giving
  dense per-edge layouts wd [128, 384] / ws [128, 24] that match the
  host-prepared dense operand arrays (bp_src, bp_dst, v).
- DVE (all dense): t_d = wd >> bp_d; t_s = ws(bcast K) >> bp_s;
  m = t_d & t_s; mask = (m << 31) >>a 31; out = mask & v_bits.
- Host scatters the dense outputs back to original edge order.
"""
import numpy as np
from contextlib import ExitStack

from concourse import bacc, mybir
from concourse import tile
from concourse.ap import AP
from concourse.bass_utils import run_bass_kernel_spmd

P = 128
N_CORES = 8
TWORDS = 31250                      # uint32 words = 1M bits
K = 16                              # edges per src cluster
B = 49152                           # edges per batch per core
NE = B // P                         # 384 edges per partition per batch
NCL = NE // K                       # 24 src clusters per partition per batch
LD = B // 8                         # 6144 per-group dst stream positions
LS = B // (8 * K)                   # 384 per-group src-cluster positions
L = LD + LS                         # 6528 fused stream length per group
LI = L // 16                        # 408 idx-tile columns

_NC_CACHE = {}


def _build(nb):
    nc = bacc.Bacc()
    u32 = mybir.dt.uint32
    i16 = mybir.dt.int16
    i32 = mybir.dt.int32
    f32 = mybir.dt.float32

    idxs_d = nc.declare_dram_parameter("idxs", [nb, P, LI], i16, isOutput=False)
    bps_d = nc.declare_dram_parameter("bps", [nb, P, NE], u32, isOutput=False)
    bpd_d = nc.declare_dram_parameter("bpd", [nb, P, NE], u32, isOutput=False)
    v_d = nc.declare_dram_parameter("vv", [nb, P, NE], f32, isOutput=False)
    ktab_d = nc.declare_dram_parameter("ktab", [P, TWORDS], u32, isOutput=False)
    out_d = nc.declare_dram_parameter("out", [nb, P, NE], f32, isOutput=True)

    shr = mybir.AluOpType.logical_shift_right
    shl = mybir.AluOpType.logical_shift_left
    asr = mybir.AluOpType.arith_shift_right
    band = mybir.AluOpType.bitwise_and

    with ExitStack() as ctx:
        tc = ctx.enter_context(tile.TileContext(nc))
        tab_pool = ctx.enter_context(tc.tile_pool(name="tab", bufs=1))
        in_pool = ctx.enter_context(tc.tile_pool(name="inp", bufs=2))
        w_pool = ctx.enter_context(tc.tile_pool(name="w", bufs=2))
        d_pool = ctx.enter_context(tc.tile_pool(name="d", bufs=2))

        # split the 15.6MB table load across queues/engines so the first
        # gather isn't serialized behind one long DMA
        table_t = tab_pool.tile([P, TWORDS], u32)
        tab_chunk = -(-TWORDS // 8)
        for ci in range(8):
            lo = ci * tab_chunk
            hi = min(TWORDS, lo + tab_chunk)
            eng = nc.sync if ci % 2 == 0 else nc.scalar
            eng.dma_start(table_t[:, lo:hi], ktab_d[:, lo:hi])

        for b in range(nb):
            idx_t = in_pool.tile([P, LI], i16, tag="idx")
            nc.sync.dma_start(idx_t[:], idxs_d[b])
            bps_t = in_pool.tile([P, NE], u32, tag="bps")
            nc.sync.dma_start(bps_t[:], bps_d[b])
            bpd_t = in_pool.tile([P, NE], u32, tag="bpd")
            nc.sync.dma_start(bpd_t[:], bpd_d[b])
            v_t = in_pool.tile([P, NE], f32, tag="v")
            nc.sync.dma_start(v_t[:], v_d[b])

            w_t = w_pool.tile([P, L], u32, tag="w")
            nc.gpsimd.ap_gather(w_t[:], table_t[:], idx_t[:],
                                channels=P, num_elems=TWORDS, d=1, num_idxs=L)

            # compaction: [8, L] strided-partition views -> dense tiles
            wd_t = d_pool.tile([P, NE], u32, tag="wd")
            nc.scalar.dma_start(
                wd_t[:],
                AP(tensor=w_t[:].tensor, offset=0, ap=[[16 * L, 8], [1, LD]]))
            ws_t = d_pool.tile([P, NCL], u32, tag="ws")
            nc.scalar.dma_start(
                ws_t[:],
                AP(tensor=w_t[:].tensor, offset=LD, ap=[[16 * L, 8], [1, LS]]))

            # t_d = wd >> bp_d
            nc.vector.tensor_tensor(wd_t[:], wd_t[:], bpd_t[:], op=shr)
            # t_s = ws(bcast K) >> bp_s
            tmp_t = d_pool.tile([P, NE], u32, tag="tmp")
            ws3 = ws_t[:].unsqueeze(2).to_broadcast([P, NCL, K])
            nc.vector.tensor_tensor(
                tmp_t[:].rearrange("p (c k) -> p c k", c=NCL, k=K),
                ws3,
                bps_t[:].rearrange("p (c k) -> p c k", c=NCL, k=K),
                op=shr)
            # m = t_d & t_s
            nc.vector.tensor_tensor(wd_t[:], wd_t[:], tmp_t[:], op=band)
            # mask = (m << 31) >>a 31  (0xFFFFFFFF iff bit0 set)
            nc.vector.tensor_scalar(wd_t[:].bitcast(i32), wd_t[:].bitcast(i32),
                                    31, 31, op0=shl, op1=asr)
            # out = mask & v_bits
            o_t = d_pool.tile([P, NE], f32, tag="o")
            nc.vector.tensor_tensor(o_t[:].bitcast(u32), wd_t[:],
                                    v_t[:].bitcast(u32), op=band)
            nc.scalar.dma_start(out_d[b], o_t[:])
    nc.finalize()
    return nc


def _prep_core(src, dst, vals, nb_hint=None):
    """Cluster one core's edges by src word; build dense operand arrays.

    Returns dict of device arrays + (ids, nb) for output unpacking."""
    e = src.shape[0]
    ws_word = (src >> 5).astype(np.int64)
    bp_s = (src & 31).astype(np.uint32)
    wd_word = (dst >> 5).astype(np.uint16)
    bp_d = (dst & 31).astype(np.uint32)

    order = np.argsort(ws_word, kind="stable")
    sw = ws_word[order]
    counts = np.bincount(sw, minlength=TWORDS)
    starts = np.zeros(TWORDS + 1, dtype=np.int64)
    np.cumsum(counts, out=starts[1:])
    within = np.arange(e, dtype=np.int64) - starts[sw]
    cl_per_word = -(-counts // K)
    cl_base = np.zeros(TWORDS + 1, dtype=np.int64)
    np.cumsum(cl_per_word, out=cl_base[1:])
    cl = cl_base[sw] + within // K          # cluster id per sorted edge
    t_in_cl = within % K
    tc_total = int(cl_base[-1])

    bins = -(-tc_total // NCL)
    nb = -(-bins // P)
    if nb_hint is not None:
        nb = max(nb, nb_hint)

    batch = cl // (P * NCL)
    part = (cl // NCL) % P
    s_cl = cl % NCL
    j = s_cl * K + t_in_cl
    slot = (batch * P + part) * NE + j

    bps = np.zeros(nb * P * NE, dtype=np.uint32)
    bpd = np.zeros(nb * P * NE, dtype=np.uint32)
    vv = np.zeros(nb * P * NE, dtype=np.float32)
    wdw = np.zeros(nb * P * NE, dtype=np.uint16)
    ids = np.full(nb * P * NE, -1, dtype=np.int64)
    bps[slot] = bp_s[order]
    bpd[slot] = bp_d[order]
    vv[slot] = vals[order]
    wdw[slot] = wd_word[order]
    ids[slot] = order

    clw = np.zeros(nb * P * NCL, dtype=np.uint16)
    cl_slot = (batch * P + part) * NCL + s_cl
    clw[cl_slot] = sw

    # fused per-group stream: [dst words (16*NE)] ++ [cluster words (16*NCL)]
    dd = wdw.reshape(nb, 8, 16 * NE)
    cw = clw.reshape(nb, 8, 16 * NCL)
    st = np.concatenate([dd, cw], axis=2)               # [nb, 8, L]
    idx_tile = (st.reshape(nb, 8, LI, 16)
                .transpose(0, 1, 3, 2)
                .reshape(nb, P, LI)
                .astype(np.int16))

    return {
        "idxs": idx_tile,
        "bps": bps.reshape(nb, P, NE),
        "bpd": bpd.reshape(nb, P, NE),
        "vv": vv.reshape(nb, P, NE),
    }, ids, nb


def prepare(inputs):
    """Build (nc, in_maps, ids_list) for the full-problem inputs dict."""
    edge_index = np.asarray(inputs["edge_index"])
    values = np.asarray(inputs["values"], dtype=np.float32)
    nodes_flag = np.asarray(inputs["nodes_flag"], dtype=bool)
    e_total = values.shape[0]
    assert e_total % N_CORES == 0
    e_per = e_total // N_CORES

    keep = ~nodes_flag
    keep_pad = np.zeros(TWORDS * 32, dtype=bool)
    keep_pad[:keep.shape[0]] = keep
    ktab_words = np.packbits(keep_pad, bitorder="little").view(np.uint32)
    ktab = np.ascontiguousarray(np.broadcast_to(ktab_words, (P, TWORDS)))

    src_all = edge_index[0].astype(np.uint32)
    dst_all = edge_index[1].astype(np.uint32)

    cores = []
    nb = 0
    for c in range(N_CORES):
        lo, hi = c * e_per, (c + 1) * e_per
        arrs, ids, nb_c = _prep_core(src_all[lo:hi], dst_all[lo:hi], values[lo:hi])
        cores.append((arrs, ids))
        nb = max(nb, nb_c)

    # all cores must share one compiled program: pad each to nb batches
    in_maps = []
    ids_list = []
    for arrs, ids in cores:
        nb_c = arrs["idxs"].shape[0]
        if nb_c < nb:
            for k in arrs:
                pad_shape = (nb - nb_c,) + arrs[k].shape[1:]
                arrs[k] = np.concatenate(
                    [arrs[k], np.zeros(pad_shape, dtype=arrs[k].dtype)], axis=0)
            ids = np.concatenate(
                [ids, np.full((nb - nb_c) * P * NE, -1, dtype=np.int64)])
        arrs["ktab"] = ktab
        in_maps.append(arrs)
        ids_list.append(ids)

    if nb not in _NC_CACHE:
        _NC_CACHE[nb] = _build(nb)
    return _NC_CACHE[nb], in_maps, ids_list


def kernel(edge_index: np.ndarray, values: np.ndarray, nodes_flag: np.ndarray) -> np.ndarray:
    e_total = values.shape[0]
    e_per = e_total // N_CORES
    nc, in_maps, ids_list = prepare({"edge_index": edge_index, "values": values,
                                     "nodes_flag": nodes_flag})

    res = run_bass_kernel_spmd(nc, in_maps, list(range(N_CORES)))

    outs = []
    for c in range(N_CORES):
        got = res.results[c]["out"].reshape(-1)
        ids = ids_list[c]
        valid = ids >= 0
        o = np.zeros(e_per, dtype=np.float32)
        o[ids[valid]] = got[valid]
        outs.append(o)
    return np.concatenate(outs)


if __name__ == "__main__":
    rng = np.random.default_rng(0)
    E = 1_000_000 * 8
    N = 1_000_000
    ei = rng.integers(0, N, size=(2, E), dtype=np.int64)
    v = rng.random(E, dtype=np.float32)
    flag = rng.random(N) < 0.1
    got = kernel(ei, v, flag)
    keep = (~flag).astype(np.float32)
    exp = v * keep[ei[0]] * keep[ei[1]]
    err = np.max(np.abs(got - exp))
    print("max abs err:", err, "CORRECT:", np.allclose(got, exp))


# revision 6
# speedup vs baseline: 1.0021x; 1.0021x over previous
"""NodeDropout kernel for 8 trn2 NeuronCores.

out[e] = values[e] * keep[src[e]] * keep[dst[e]],  keep = ~nodes_flag (1M bools).

Device algorithm (edges sharded 8 ways data-parallel, per core):
- keep bit-packed host-side into a 31250-word uint32 table (1M bits),
  replicated into every SBUF partition (~122KB/partition).
- gpsimd.ap_gather: each 16-partition group consumes one shared int16 index
  stream; out[p, i] = table[p, stream_{p//16}[i]] (replicated within a group
  since the table is replicated).
- Per batch of B = 49152 edges: one fused gather stream per group =
  [per-edge dst word idxs (6144)] ++ [src cluster word idxs (384)].
  Host sorts edges by src word and forms clusters of K=16 edges sharing one
  src word, so src lookups cost 1/16 per edge.
- Compaction: the gather output [128, L] is 16x redundant; a plain DMA from
  the strided-partition view [8, L] (partitions 0,16,..,112) reshaped to
  [128, L/16] lands chunk r of group c's stream on partition 16c+r - giving
  dense per-edge layouts wd [128, 384] / ws [128, 24] that match the
  host-prepared dense operand arrays (bp_src, bp_dst, v).
- DVE (all dense): t_d = wd >> bp_d; t_s = ws(bcast K) >> bp_s;
  m = t_d & t_s; mask = (m << 31) >>a 31; out = mask & v_bits.
- Host scatters the dense outputs back to original edge order.
"""
import numpy as np
from contextlib import ExitStack

from concourse import bacc, mybir
from concourse import tile
from concourse.ap import AP
from concourse.bass_utils import run_bass_kernel_spmd

P = 128
N_CORES = 8
TWORDS = 31250                      # uint32 words = 1M bits
K = 16                              # edges per src cluster
B = 49152                           # edges per batch per core
NE = B // P                         # 384 edges per partition per batch
NCL = NE // K                       # 24 src clusters per partition per batch
LD = B // 8                         # 6144 per-group dst stream positions
LS = B // (8 * K)                   # 384 per-group src-cluster positions
L = LD + LS                         # 6528 fused stream length per group
LI = L // 16                        # 408 idx-tile columns

_NC_CACHE = {}


def _build(nb):
    nc = bacc.Bacc()
    u32 = mybir.dt.uint32
    i16 = mybir.dt.int16
    i32 = mybir.dt.int32
    f32 = mybir.dt.float32

    idxs_d = nc.declare_dram_parameter("idxs", [nb, P, LI], i16, isOutput=False)
    bps_d = nc.declare_dram_parameter("bps", [nb, P, NE], u32, isOutput=False)
    bpd_d = nc.declare_dram_parameter("bpd", [nb, P, NE], u32, isOutput=False)
    v_d = nc.declare_dram_parameter("vv", [nb, P, NE], f32, isOutput=False)
    ktab_d = nc.declare_dram_parameter("ktab", [P, TWORDS], u32, isOutput=False)
    out_d = nc.declare_dram_parameter("out", [nb, P, NE], f32, isOutput=True)

    shr = mybir.AluOpType.logical_shift_right
    shl = mybir.AluOpType.logical_shift_left
    asr = mybir.AluOpType.arith_shift_right
    band = mybir.AluOpType.bitwise_and

    with ExitStack() as ctx:
        tc = ctx.enter_context(tile.TileContext(nc))
        tab_pool = ctx.enter_context(tc.tile_pool(name="tab", bufs=1))
        in_pool = ctx.enter_context(tc.tile_pool(name="inp", bufs=2))
        w_pool = ctx.enter_context(tc.tile_pool(name="w", bufs=2))
        d_pool = ctx.enter_context(tc.tile_pool(name="d", bufs=2))

        # split the 15.6MB table load across queues/engines so the first
        # gather isn't serialized behind one long DMA
        table_t = tab_pool.tile([P, TWORDS], u32)
        tab_chunk = -(-TWORDS // 8)
        for ci in range(8):
            lo = ci * tab_chunk
            hi = min(TWORDS, lo + tab_chunk)
            eng = nc.sync if ci % 2 == 0 else nc.scalar
            eng.dma_start(table_t[:, lo:hi], ktab_d[:, lo:hi])

        for b in range(nb):
            idx_t = in_pool.tile([P, LI], i16, tag="idx")
            nc.sync.dma_start(idx_t[:], idxs_d[b])
            bps_t = in_pool.tile([P, NE], u32, tag="bps")
            nc.sync.dma_start(bps_t[:], bps_d[b])
            bpd_t = in_pool.tile([P, NE], u32, tag="bpd")
            nc.sync.dma_start(bpd_t[:], bpd_d[b])
            v_t = in_pool.tile([P, NE], f32, tag="v")
            nc.sync.dma_start(v_t[:], v_d[b])

            w_t = w_pool.tile([P, L], u32, tag="w")
            nc.gpsimd.ap_gather(w_t[:], table_t[:], idx_t[:],
                                channels=P, num_elems=TWORDS, d=1, num_idxs=L)

            # compaction: [8, L] strided-partition views -> dense tiles
            wd_t = d_pool.tile([P, NE], u32, tag="wd")
            nc.scalar.dma_start(
                wd_t[:],
                AP(tensor=w_t[:].tensor, offset=0, ap=[[16 * L, 8], [1, LD]]))
            ws_t = d_pool.tile([P, NCL], u32, tag="ws")
            nc.scalar.dma_start(
                ws_t[:],
                AP(tensor=w_t[:].tensor, offset=LD, ap=[[16 * L, 8], [1, LS]]))

            # t_d = wd >> bp_d
            nc.vector.tensor_tensor(wd_t[:], wd_t[:], bpd_t[:], op=shr)
            # t_s = ws(bcast K) >> bp_s
            tmp_t = d_pool.tile([P, NE], u32, tag="tmp")
            ws3 = ws_t[:].unsqueeze(2).to_broadcast([P, NCL, K])
            nc.vector.tensor_tensor(
                tmp_t[:].rearrange("p (c k) -> p c k", c=NCL, k=K),
                ws3,
                bps_t[:].rearrange("p (c k) -> p c k", c=NCL, k=K),
                op=shr)
            # m = t_d & t_s
            nc.vector.tensor_tensor(wd_t[:], wd_t[:], tmp_t[:], op=band)
            # mask = (m << 31) >>a 31  (0xFFFFFFFF iff bit0 set)
            nc.vector.tensor_scalar(wd_t[:].bitcast(i32), wd_t[:].bitcast(i32),
                                    31, 31, op0=shl, op1=asr)
            # out = mask & v_bits
            o_t = d_pool.tile([P, NE], f32, tag="o")
            nc.vector.tensor_tensor(o_t[:].bitcast(u32), wd_t[:],
                                    v_t[:].bitcast(u32), op=band)
            nc.scalar.dma_start(out_d[b], o_t[:])
    nc.finalize()
    return nc


def _prep_core(src, dst, vals, nb_hint=None):
    """Cluster one core's edges by src word; build dense operand arrays.

    Returns dict of device arrays + (ids, nb) for output unpacking."""
    e = src.shape[0]
    ws_word = (src >> 5).astype(np.int64)
    bp_s = (src & 31).astype(np.uint32)
    wd_word = (dst >> 5).astype(np.uint16)
    bp_d = (dst & 31).astype(np.uint32)

    order = np.argsort(ws_word, kind="stable")
    sw = ws_word[order]
    counts = np.bincount(sw, minlength=TWORDS)
    starts = np.zeros(TWORDS + 1, dtype=np.int64)
    np.cumsum(counts, out=starts[1:])
    within = np.arange(e, dtype=np.int64) - starts[sw]
    cl_per_word = -(-counts // K)
    cl_base = np.zeros(TWORDS + 1, dtype=np.int64)
    np.cumsum(cl_per_word, out=cl_base[1:])
    cl = cl_base[sw] + within // K          # cluster id per sorted edge
    t_in_cl = within % K
    tc_total = int(cl_base[-1])

    bins = -(-tc_total // NCL)
    nb = -(-bins // P)
    if nb_hint is not None:
        nb = max(nb, nb_hint)

    batch = cl // (P * NCL)
    part = (cl // NCL) % P
    s_cl = cl % NCL
    j = s_cl * K + t_in_cl
    slot = (batch * P + part) * NE + j

    bps = np.zeros(nb * P * NE, dtype=np.uint32)
    bpd = np.zeros(nb * P * NE, dtype=np.uint32)
    vv = np.zeros(nb * P * NE, dtype=np.float32)
    wdw = np.zeros(nb * P * NE, dtype=np.uint16)
    ids = np.full(nb * P * NE, -1, dtype=np.int64)
    bps[slot] = bp_s[order]
    bpd[slot] = bp_d[order]
    vv[slot] = vals[order]
    wdw[slot] = wd_word[order]
    ids[slot] = order

    clw = np.zeros(nb * P * NCL, dtype=np.uint16)
    cl_slot = (batch * P + part) * NCL + s_cl
    clw[cl_slot] = sw

    # fused per-group stream: [dst words (16*NE)] ++ [cluster words (16*NCL)]
    dd = wdw.reshape(nb, 8, 16 * NE)
    cw = clw.reshape(nb, 8, 16 * NCL)
    st = np.concatenate([dd, cw], axis=2)               # [nb, 8, L]
    idx_tile = (st.reshape(nb, 8, LI, 16)
                .transpose(0, 1, 3, 2)
                .reshape(nb, P, LI)
                .astype(np.int16))

    return {
        "idxs": idx_tile,
        "bps": bps.reshape(nb, P, NE),
        "bpd": bpd.reshape(nb, P, NE),
        "vv": vv.reshape(nb, P, NE),
    }, ids, nb


def prepare(inputs):
    """Build (nc, in_maps, ids_list) for the full-problem inputs dict."""
    edge_index = np.asarray(inputs["edge_index"])
    values = np.asarray(inputs["values"], dtype=np.float32)
    nodes_flag = np.asarray(inputs["nodes_flag"], dtype=bool)
    e_total = values.shape[0]
    assert e_total % N_CORES == 0
    e_per = e_total // N_CORES

    keep = ~nodes_flag
    keep_pad = np.zeros(TWORDS * 32, dtype=bool)
    keep_pad[:keep.shape[0]] = keep
    ktab_words = np.packbits(keep_pad, bitorder="little").view(np.uint32)
    ktab = np.ascontiguousarray(np.broadcast_to(ktab_words, (P, TWORDS)))

    src_all = edge_index[0].astype(np.uint32)
    dst_all = edge_index[1].astype(np.uint32)

    cores = []
    nb = 0
    for c in range(N_CORES):
        lo, hi = c * e_per, (c + 1) * e_per
        arrs, ids, nb_c = _prep_core(src_all[lo:hi], dst_all[lo:hi], values[lo:hi])
        cores.append((arrs, ids))
        nb = max(nb, nb_c)

    # all cores must share one compiled program: pad each to nb batches
    in_maps = []
    ids_list = []
    for arrs, ids in cores:
        nb_c = arrs["idxs"].shape[0]
        if nb_c < nb:
            for k in arrs:
                pad_shape = (nb - nb_c,) + arrs[k].shape[1:]
                arrs[k] = np.concatenate(
                    [arrs[k], np.zeros(pad_shape, dtype=arrs[k].dtype)], axis=0)
            ids = np.concatenate(
                [ids, np.full((nb - nb_c) * P * NE, -1, dtype=np.int64)])
        arrs["ktab"] = ktab
        in_maps.append(arrs)
        ids_list.append(ids)

    if nb not in _NC_CACHE:
        _NC_CACHE[nb] = _build(nb)
    return _NC_CACHE[nb], in_maps, ids_list


def kernel(edge_index: np.ndarray, values: np.ndarray, nodes_flag: np.ndarray) -> np.ndarray:
    e_total = values.shape[0]
    e_per = e_total // N_CORES
    nc, in_maps, ids_list = prepare({"edge_index": edge_index, "values": values,
                                     "nodes_flag": nodes_flag})

    res = run_bass_kernel_spmd(nc, in_maps, list(range(N_CORES)))

    outs = []
    for c in range(N_CORES):
        got = res.results[c]["out"].reshape(-1)
        ids = ids_list[c]
        valid = ids >= 0
        o = np.zeros(e_per, dtype=np.float32)
        o[ids[valid]] = got[valid]
        outs.append(o)
    return np.concatenate(outs)


if __name__ == "__main__":
    rng = np.random.default_rng(0)
    E = 1_000_000 * 8
    N = 1_000_000
    ei = rng.integers(0, N, size=(2, E), dtype=np.int64)
    v = rng.random(E, dtype=np.float32)
    flag = rng.random(N) < 0.1
    got = kernel(ei, v, flag)
    keep = (~flag).astype(np.float32)
    exp = v * keep[ei[0]] * keep[ei[1]]
    err = np.max(np.abs(got - exp))
    print("max abs err:", err, "CORRECT:", np.allclose(got, exp))
